# revision 8
# baseline (speedup 1.0000x reference)
"""Trainium2 Bass kernel for DecoderCrossAttention (B=8, S=2048, T=1024, E=1024, C=768, H=16, D=64).

Data-parallel over batch: 8 NeuronCores, one batch element each, no collectives.

Per-core layout ("transposed" so every matmul has its contraction dim on SBUF partitions):
    xT [E,S], qT [E,S] (bf16), kT [E,T] (bf16): E' rows are head-major (head h = rows 64h..64h+63)
    scoresT_h [T, S-chunk] via K=64 matmuls, two heads row-packed in the PE array
    eT_h = exp(scoresT_h / 8) in one ACT pass (scores are O(1): no max subtraction needed),
           bf16, with row sums via ones-matmuls (col-packed, M=32 replicated rows)
    outT_h [D, S] = (eT_h^T-contraction) @ v0_h, col-packed head pairs, normalized by 1/sum
    avg_attn = (1/H) Sum_h eT_h * (1/s_h): bf16 DVE accumulate, PE-transposed back to [S, T]
    out = outT^T @ Wo(bf16) + bo_eff,  bo_eff = bv @ Wo + bo computed on device
"""

import sys

sys.path.insert(0, "/opt/trn_rl_repo")

from contextlib import ExitStack

import numpy as np

import concourse.bass as bass
import concourse.mybir as mybir
import concourse.tile as tile
from concourse import bacc
from concourse.bass_utils import run_bass_kernel_spmd
from concourse.masks import make_identity

F32 = mybir.dt.float32
F32R = mybir.dt.float32r
BF16 = mybir.dt.bfloat16
AF = mybir.ActivationFunctionType
OP = mybir.AluOpType

N_CORES = 8
S, T, E, C = 2048, 1024, 1024, 768
H, D = 16, 64
P = 128
SC = 256  # S-chunk size
NCH = S // SC
KE = E // P  # 8
KC = C // P  # 6
TT = T // P  # 8
SCALE = 0.125

_PROGRAM = None


def r32(ap):
    return ap.bitcast(F32R)



def _bcast_dma(nc, out_t, src_row):
    """Broadcast a [1, N] SBUF row to [P, N] via DMA (zero-step free dim on the source)."""
    src_b = bass.AP(
        tensor=src_row.tensor,
        offset=src_row.offset,
        ap=[list(src_row.ap[0]), [0, out_t.shape[0]]] + [list(d) for d in src_row.ap[1:]],
    )
    nc.sync.dma_start(out=out_t, in_=src_b)

def build_program():
    nc = bacc.Bacc("TRN2", target_bir_lowering=False, debug=False, num_devices=N_CORES)

    x = nc.dram_tensor("x", [S, E], F32, kind="ExternalInput").ap()
    enc = nc.dram_tensor("enc", [T, C], F32, kind="ExternalInput").ap()
    Wq = nc.dram_tensor("Wq", [E, E], F32, kind="ExternalInput").ap()
    bq = nc.dram_tensor("bq", [E], F32, kind="ExternalInput").ap()
    Wk = nc.dram_tensor("Wk", [C, E], F32, kind="ExternalInput").ap()
    bk = nc.dram_tensor("bk", [E], F32, kind="ExternalInput").ap()
    Wv = nc.dram_tensor("Wv", [C, E], F32, kind="ExternalInput").ap()
    bv = nc.dram_tensor("bv", [E], F32, kind="ExternalInput").ap()
    Wo = nc.dram_tensor("Wo", [E, E], F32, kind="ExternalInput").ap()
    bo = nc.dram_tensor("bo", [E], F32, kind="ExternalInput").ap()
    out = nc.dram_tensor("out", [S, E], F32, kind="ExternalOutput").ap()
    avg = nc.dram_tensor("avg", [S, T], F32, kind="ExternalOutput").ap()

    with tile.TileContext(nc) as tc:
        _build(tc, x, enc, Wq, bq, Wk, bk, Wv, bv, Wo, bo, out, avg)
    nc.compile()
    return nc


def _build(tc, x, enc, Wq, bq, Wk, bk, Wv, bv, Wo, bo, out, avg):
    nc = tc.nc
    with ExitStack() as stack:
        consts = stack.enter_context(tc.tile_pool(name="consts", bufs=1))
        resident = stack.enter_context(tc.tile_pool(name="resident", bufs=1))

        ident_f = consts.tile([P, P], F32)
        make_identity(nc, ident_f)
        ident_b = consts.tile([P, P], BF16)
        make_identity(nc, ident_b)
        ones32 = consts.tile([P, 32], BF16)
        nc.vector.memset(ones32, 1.0)
        bq_sb = consts.tile([P, KE], F32)
        nc.sync.dma_start(out=bq_sb, in_=bq.rearrange("(m p) -> p m", p=P))
        bk_sb = consts.tile([P, KE], F32)
        nc.sync.dma_start(out=bk_sb, in_=bk.rearrange("(m p) -> p m", p=P))
        bv_sb = consts.tile([P, KE], F32)
        nc.sync.dma_start(out=bv_sb, in_=bv.rearrange("(m p) -> p m", p=P))
        bo_row = consts.tile([1, E], F32)
        nc.sync.dma_start(out=bo_row, in_=bo[None, :])
        bo_bcast = consts.tile([P, E], F32)

        Wq_sb = resident.tile([P, KE, E], F32)
        nc.sync.dma_start(out=Wq_sb.bitcast(F32R), in_=Wq.rearrange("(k p) n -> p k n", p=P).bitcast(F32R))
        Wo_bf = resident.tile([P, KE, E], BF16)
        kT_bf = resident.tile([P, KE, T], BF16)
        v0_sb = resident.tile([P, TT, E], BF16)

        # ---------------- phase A ----------------
        with ExitStack() as ph:
            early = ph.enter_context(tc.tile_pool(name="early", bufs=1))
            enc_pool = ph.enter_context(tc.tile_pool(name="enc_pool", bufs=2))
            wo_pool = ph.enter_context(tc.tile_pool(name="wo_pool", bufs=2))
            ph_ps = ph.enter_context(tc.tile_pool(name="ph_ps", bufs=2, space="PSUM"))

            Wk_sb = early.tile([P, KC, E], F32)
            nc.sync.dma_start(out=Wk_sb.bitcast(F32R), in_=Wk.rearrange("(k p) n -> p k n", p=P).bitcast(F32R))
            Wv_sb = early.tile([P, KC, E], F32)
            nc.sync.dma_start(out=Wv_sb.bitcast(F32R), in_=Wv.rearrange("(k p) n -> p k n", p=P).bitcast(F32R))
            encT_sb = early.tile([P, KC, T], F32)

            for t8 in range(TT):
                enc_t = enc_pool.tile([P, C], F32, tag="enc_t")
                nc.sync.dma_start(out=enc_t, in_=enc[t8 * P : (t8 + 1) * P, :])
                ps_a = ph_ps.tile([P, E], F32, tag="ph")
                for c6 in range(KC):
                    nc.tensor.transpose(
                        ps_a[:, c6 * P : (c6 + 1) * P],
                        enc_t[:, c6 * P : (c6 + 1) * P],
                        ident_f,
                    )
                nc.vector.tensor_copy(
                    encT_sb[:, :, t8 * P : (t8 + 1) * P].bitcast(F32R),
                    ps_a[:, : KC * P].rearrange("p (k t) -> p k t", k=KC),
                )

            for k8 in range(KE):
                wo_t = wo_pool.tile([P, E], F32, tag="wo_t")
                nc.sync.dma_start(out=wo_t, in_=Wo[k8 * P : (k8 + 1) * P, :])
                nc.vector.tensor_copy(Wo_bf[:, k8, :], wo_t)

            # kT[e',t] = sum_c Wk[c,e'] encT[c,t], +bk, store bf16
            for m8 in range(KE):
                ps_a = ph_ps.tile([P, T], F32, tag="ph")
                for c6 in range(KC):
                    for n2 in range(2):
                        nc.tensor.matmul(
                            ps_a[:, n2 * 512 : (n2 + 1) * 512],
                            r32(Wk_sb[:, c6, m8 * P : (m8 + 1) * P]),
                            r32(encT_sb[:, c6, n2 * 512 : (n2 + 1) * 512]),
                            start=(c6 == 0),
                            stop=(c6 == KC - 1),
                        )
                nc.scalar.activation(
                    kT_bf[:, m8, :], ps_a, AF.Identity, bias=bk_sb[:, m8 : m8 + 1]
                )

            # v0[t,e'] = sum_c encT[c,t] Wv[c,e'] (bias folded into bo_eff)
            for t8 in range(TT):
                ps_a = ph_ps.tile([P, E], F32, tag="ph")
                for c6 in range(KC):
                    for n2 in range(2):
                        nc.tensor.matmul(
                            ps_a[:, n2 * 512 : (n2 + 1) * 512],
                            r32(encT_sb[:, c6, t8 * P : (t8 + 1) * P]),
                            r32(Wv_sb[:, c6, n2 * 512 : (n2 + 1) * 512]),
                            start=(c6 == 0),
                            stop=(c6 == KC - 1),
                        )
                nc.vector.tensor_copy(v0_sb[:, t8, :], ps_a)

            # bo_eff = bv @ Wo + bo
            bv_bf = early.tile([P, KE], BF16)
            nc.vector.tensor_copy(bv_bf, bv_sb)
            ps_a = ph_ps.tile([1, E], F32, tag="ph")
            for k8 in range(KE):
                for n2 in range(2):
                    nc.tensor.matmul(
                        ps_a[:, n2 * 512 : (n2 + 1) * 512],
                        bv_bf[:, k8 : k8 + 1],
                        Wo_bf[:, k8, n2 * 512 : (n2 + 1) * 512],
                        start=(k8 == 0),
                        stop=(k8 == KE - 1),
                    )
            bo_eff = early.tile([1, E], F32)
            nc.vector.tensor_tensor(bo_eff, ps_a[:1, :], bo_row, OP.add)
            _bcast_dma(nc, bo_bcast, bo_eff)

        # ---------------- phase B: S-chunk loop ----------------
        with ExitStack() as mn:
            xload = mn.enter_context(tc.tile_pool(name="xload", bufs=1))
            xt_pool = mn.enter_context(tc.tile_pool(name="xt_pool", bufs=1))
            qt_pool = mn.enter_context(tc.tile_pool(name="qt_pool", bufs=2))
            et_pool = mn.enter_context(tc.tile_pool(name="et_pool", bufs=3))
            tmp_pool = mn.enter_context(tc.tile_pool(name="tmp_pool", bufs=2))
            acc_pool = mn.enter_context(tc.tile_pool(name="acc_pool", bufs=2))
            ot_pool = mn.enter_context(tc.tile_pool(name="ot_pool", bufs=2))
            wb_pool = mn.enter_context(tc.tile_pool(name="wb_pool", bufs=5))
            st_pool = mn.enter_context(tc.tile_pool(name="st_pool", bufs=2))
            out_pool = mn.enter_context(tc.tile_pool(name="out_pool", bufs=2))

            ps_scores = mn.enter_context(
                tc.tile_pool(name="ps_scores", bufs=1, space="PSUM")
            )
            ps_outp = mn.enter_context(tc.tile_pool(name="ps_outp", bufs=2, space="PSUM"))
            ps_sums = mn.enter_context(tc.tile_pool(name="ps_sums", bufs=1, space="PSUM"))
            ps_misc = mn.enter_context(tc.tile_pool(name="ps_misc", bufs=1, space="PSUM"))

            for c in range(NCH):
                s0 = c * SC
                x_sb = xload.tile([P, 2, E], F32)
                nc.sync.dma_start(
                    out=x_sb, in_=x[s0 : s0 + SC, :].rearrange("(a p) e -> p a e", p=P)
                )
                xT_c = xt_pool.tile([P, KE, SC], F32)
                for e8 in range(KE):
                    ps_m = ps_misc.tile([P, 512], F32, tag="misc")
                    for s2 in range(2):
                        nc.tensor.transpose(
                            ps_m[:, s2 * P : (s2 + 1) * P],
                            x_sb[:, s2, e8 * P : (e8 + 1) * P],
                            ident_f,
                        )
                    nc.vector.tensor_copy(xT_c[:, e8, :].bitcast(F32R), ps_m[:, :SC])

                qT_c = qt_pool.tile([P, KE, SC], BF16)
                for m8 in range(KE):
                    ps_m = ps_misc.tile([P, 512], F32, tag="misc")
                    for k8 in range(KE):
                        nc.tensor.matmul(
                            ps_m[:, :SC],
                            r32(Wq_sb[:, k8, m8 * P : (m8 + 1) * P]),
                            r32(xT_c[:, k8, :]),
                            start=(k8 == 0),
                            stop=(k8 == KE - 1),
                        )
                    nc.scalar.activation(
                        qT_c[:, m8, :], ps_m[:, :SC], AF.Identity,
                        bias=bq_sb[:, m8 : m8 + 1],
                    )

                acc = acc_pool.tile([P, TT, SC], BF16)
                outT = ot_pool.tile([P, KE, SC], BF16)

                for q in range(H // 4):  # quads of 4 heads
                    ps_s = ps_sums.tile([P, SC], F32, tag="sums")
                    po_q = {}
                    eT_q = {}
                    for pp in range(2):  # pairs within quad
                        pr = 2 * q + pp
                        hA, hB = 2 * pr, 2 * pr + 1
                        eT_pair = et_pool.tile([P, 2, TT, SC], BF16, tag="eT", name="eT_pair")
                        for half in range(2):
                            ps_sc = ps_scores.tile([P, 8, SC], F32, tag="scores", name="ps_sc")
                            for t4 in range(4):
                                t8 = half * 4 + t4
                                for hh, tp in ((0, 0), (1, 64)):
                                    nc.tensor.matmul(
                                        ps_sc[:, 4 * hh + t4, :],
                                        kT_bf[tp : tp + D, pr, t8 * P : (t8 + 1) * P],
                                        qT_c[tp : tp + D, pr, :],
                                        start=True,
                                        stop=True,
                                        tile_position=(tp, 0),
                                    )
                            nc.scalar.activation(
                                eT_pair[:, :, half * 4 : half * 4 + 4, :],
                                ps_sc,
                                AF.Exp,
                                scale=SCALE,
                            )
                        po = ps_outp.tile([P, SC], F32, tag="po")
                        for hh, tp in ((0, 0), (1, 64)):
                            h = 2 * pr + hh
                            for t8 in range(TT):
                                nc.tensor.matmul(
                                    po[tp : tp + D, :],
                                    v0_sb[:, t8, h * D : (h + 1) * D],
                                    eT_pair[:, hh, t8, :],
                                    start=(t8 == 0),
                                    stop=(t8 == TT - 1),
                                    tile_position=(0, tp),
                                )
                        for hh in range(2):
                            h = 2 * pr + hh
                            q32 = 32 * (h % 4)
                            for t8 in range(TT):
                                nc.tensor.matmul(
                                    ps_s[q32 : q32 + 32, :],
                                    ones32,
                                    eT_pair[:, hh, t8, :],
                                    start=(t8 == 0),
                                    stop=(t8 == TT - 1),
                                    tile_position=(0, q32),
                                )
                        po_q[pp] = po
                        eT_q[pr] = eT_pair

                    # quad complete: reciprocal of sums, broadcast, normalize, accumulate
                    rq = st_pool.tile([P, SC], F32, tag="rq")
                    nc.vector.reciprocal_approx_fast(out=rq, in_=ps_s)
                    rqb = st_pool.tile([P, SC], BF16, tag="rqb")
                    nc.vector.tensor_copy(rqb, rq)

                    wb = {}
                    for j in range(4):
                        h = 4 * q + j
                        wb[h] = wb_pool.tile([P, SC], BF16, tag="wb", name="wb_t")
                        _bcast_dma(nc, wb[h], rqb[32 * j : 32 * j + 1, :])

                    for pp in range(2):
                        pr = 2 * q + pp
                        wbp = wb_pool.tile([P, SC], BF16, tag="wbp", name="wbp_t")
                        _bcast_dma(nc, wbp[0:64, :], rqb[32 * (2 * pp) : 32 * (2 * pp) + 1, :])
                        _bcast_dma(nc, wbp[64:128, :], rqb[32 * (2 * pp + 1) : 32 * (2 * pp + 1) + 1, :])
                        nc.vector.tensor_tensor(outT[:, pr, :], po_q[pp], wbp, OP.mult)

                    for j in range(4):
                        h = 4 * q + j
                        pr, hh = divmod(h, 2)
                        eT_h = eT_q[pr][:, hh]
                        wb_b = wb[h][:, None, :].to_broadcast([P, TT, SC])
                        if h == 0:
                            nc.vector.tensor_tensor(acc, eT_h, wb_b, OP.mult)
                        else:
                            tmp = tmp_pool.tile([P, TT, SC], BF16, tag="tmp")
                            nc.vector.tensor_tensor(tmp, eT_h, wb_b, OP.mult)
                            nc.vector.tensor_tensor(acc, acc, tmp, OP.add)

                # final projection
                for m2 in range(2):
                    out_sb = out_pool.tile([P, E], F32, tag="out_sb")
                    for n2 in range(2):
                        ps_m = ps_misc.tile([P, 512], F32, tag="misc")
                        for k8 in range(KE):
                            nc.tensor.matmul(
                                ps_m,
                                outT[:, k8, m2 * P : (m2 + 1) * P],
                                Wo_bf[:, k8, n2 * 512 : (n2 + 1) * 512],
                                start=(k8 == 0),
                                stop=(k8 == KE - 1),
                            )
                        nc.vector.tensor_tensor(
                            out_sb[:, n2 * 512 : (n2 + 1) * 512],
                            ps_m,
                            bo_bcast[:, n2 * 512 : (n2 + 1) * 512],
                            OP.add,
                        )
                    nc.sync.dma_start(
                        out=out[s0 + m2 * P : s0 + (m2 + 1) * P, :], in_=out_sb
                    )

                # avg_attn: PE-transpose acc back to [s, t], scale by 1/H
                for s2 in range(2):
                    avg_sb = out_pool.tile([P, T], F32, tag="avg_sb")
                    for th in range(2):
                        ps_m = ps_misc.tile([P, 512], BF16, tag="misc")
                        for t4 in range(4):
                            t8 = th * 4 + t4
                            nc.tensor.transpose(
                                ps_m[:, t4 * P : (t4 + 1) * P],
                                acc[:, t8, s2 * P : (s2 + 1) * P],
                                ident_b,
                            )
                        nc.vector.tensor_scalar(
                            avg_sb[:, th * 512 : (th + 1) * 512],
                            ps_m,
                            1.0 / H,
                            None,
                            OP.mult,
                        )
                    nc.sync.dma_start(
                        out=avg[s0 + s2 * P : s0 + (s2 + 1) * P, :], in_=avg_sb
                    )


def get_program():
    global _PROGRAM
    if _PROGRAM is None:
        _PROGRAM = build_program()
    return _PROGRAM


def kernel(**inputs):
    nc = get_program()
    common = {
        k: np.ascontiguousarray(np.asarray(inputs[k], dtype=np.float32))
        for k in ("Wq", "bq", "Wk", "bk", "Wv", "bv", "Wo", "bo")
    }
    x = np.asarray(inputs["x"], dtype=np.float32)
    enc = np.asarray(inputs["encoder_output"], dtype=np.float32)
    in_maps = [
        dict(common, x=np.ascontiguousarray(x[b]), enc=np.ascontiguousarray(enc[b]))
        for b in range(N_CORES)
    ]
    res = run_bass_kernel_spmd(nc, in_maps, list(range(N_CORES)))
    out = np.stack([np.asarray(res.results[b]["out"]) for b in range(N_CORES)])
    avg = np.stack([np.asarray(res.results[b]["avg"]) for b in range(N_CORES)])
    return out, avg
